# revision 1
# baseline (speedup 1.0000x reference)
"""Trainium2 Bass kernel for PoissonGaussianReadout.

Computation (per reference):
  out[b, n] = elu( sum_c bilinear_sample(x[b, c], mu[n]) * W[n, c] + bias[n] ) + 1

Sharding: data-parallel over batch B=32 across 8 cores (4 images per core).
Every core processes all N=8192 neurons for its 4 images.

Device strategy per core:
  - x is pre-transposed (host) to pixel-major x_t[4096, 4*256] bf16 so that one
    pixel's (b, c) values form a contiguous 2KB row.
  - For each tile of 128 neurons, gather the 4 bilinear corner pixel-rows as
    two overlapping row-pairs (x0, x0+1) at y0 and y1 via dma_gather
    (int16 indices, elem_size=2048, elem_step=1024) -> G[128, y2, x2, b, c].
  - V[n, k, c] = w_k[n] * W[n, c] is host-precomputed (bf16, resident 16MB).
  - Per b: one fused tensor_tensor_reduce: accum z[n, b] = bias[n] +
    sum_{k,c} G * V.
  - Epilogue: out = exp(min(z,0)) + max(z,0)  (== elu(z) + 1).
"""

import dataclasses

import numpy as np
import ml_dtypes

B, C, H, Wd, N = 32, 256, 64, 64, 8192
NCORES = 8
BL = B // NCORES          # 4 images per core
P = 128                   # partitions / neurons per tile
NT = N // P               # 64 neuron tiles
ROW = BL * C              # 1024 elements per pixel row
NPIX = H * Wd             # 4096

GATHER = "ant"            # "ant" (dma_gather) or "indirect"
COMPUTE = "stt"           # "stt" (scalar_tensor_tensor; "ttr" crashes HW)
TG = 2                    # tiles per dma_gather instruction
GBUFS = 2                 # gather pool buffers
NQUEUES = 1               # SWDGE queues

_PROGRAM = None


def _build_program(nt=NT, gather=None, compute=None, tg=None, gbufs=None,
                   nqueues=None, split_waits=True):
    import concourse.bass as bass
    import concourse.mybir as mybir
    import concourse.tile as tile

    gather = gather or GATHER
    compute = compute or COMPUTE
    tg = tg or TG
    gbufs = gbufs or GBUFS
    nqueues = nqueues or NQUEUES

    bf16 = mybir.dt.bfloat16
    f32 = mybir.dt.float32
    i32 = mybir.dt.int32
    i16 = mybir.dt.int16

    nc = bass.Bass("TRN2", num_swdge_queues=nqueues)

    xt = nc.dram_tensor("xt", [NPIX, ROW], bf16, kind="ExternalInput")
    # V[n, k, c] = corner_weight_k[n] * W[n, c], host-precomputed, bf16
    wv = nc.dram_tensor("wv", [P, nt * 4 * C], bf16, kind="ExternalInput")
    biasr = nc.dram_tensor("biasr", [P, nt], f32, kind="ExternalInput")
    out = nc.dram_tensor("out", [P, nt * BL], f32, kind="ExternalOutput")
    if gather == "ant":
        idx16 = nc.dram_tensor("idx16", [P, nt * 16], i16, kind="ExternalInput")
    else:
        idx = nc.dram_tensor("idx", [P, nt * 2], i32, kind="ExternalInput")

    assert nt % tg == 0

    with tile.TileContext(nc) as tc:
        with (
            tc.tile_pool(name="const", bufs=1) as cpool,
            tc.tile_pool(name="gpool", bufs=gbufs) as gpool,
            tc.tile_pool(name="work", bufs=3) as wpool,
        ):
            if gather == "ant":
                from concourse import library_config
                nc.gpsimd.load_library(library_config.mlp)
            v_sb = cpool.tile([P, nt * 4 * C], bf16)
            nc.sync.dma_start(v_sb[:], wv[:])
            bias_sb = cpool.tile([P, nt], f32)
            nc.sync.dma_start(bias_sb[:], biasr[:])
            if gather == "ant":
                idx_sb = cpool.tile([P, nt * 16], i16)
                nc.sync.dma_start(idx_sb[:], idx16[:])
            else:
                idx_sb = cpool.tile([P, nt * 2], i32)
                nc.sync.dma_start(idx_sb[:], idx[:])
            z_sb = cpool.tile([P, nt * BL], f32)

            # DVE-side join: absorb the const-load DMA waits once, so no
            # compute instruction ever carries >1 sync wait (HW limit).
            join = cpool.tile([P, 2], f32)
            nc.vector.tensor_copy(join[:, 0:1], v_sb[:, 0:1])
            nc.vector.tensor_copy(join[:, 1:2], bias_sb[:, 0:1])

            # overlapping view of xt: row-pair i = rows (i, i+1) = 2048 elems
            xt_pairs = dataclasses.replace(
                xt[:], ap=[[ROW, NPIX - 1], [1, 2 * ROW]]
            )

            half = 2 * BL * C  # one y-row pair: [x(2), b, c]
            for tg0 in range(0, nt, tg):
                g = gpool.tile([P, tg * 4 * BL * C], bf16, tag="g")
                if gather == "ant":
                    nc.gpsimd.dma_gather(
                        out_ap=g[:].rearrange(
                            "p (s e) -> p s e", s=2 * tg, e=2048
                        ),
                        in_ap=xt_pairs,
                        idxs_ap=idx_sb[:, tg0 * 16 : (tg0 + tg) * 16],
                        num_idxs=tg * 256,
                        num_idxs_reg=tg * 256,
                        elem_size=2048,
                        elem_step=ROW,
                    )
                else:
                    for tt in range(tg):
                        t = tg0 + tt
                        for j in range(2):
                            nc.gpsimd.indirect_dma_start(
                                out=g[:, (2 * tt + j) * half : (2 * tt + j + 1) * half],
                                out_offset=None,
                                in_=xt[:, :],
                                in_offset=bass.IndirectOffsetOnAxis(
                                    ap=idx_sb[:, 2 * t + j : 2 * t + j + 1],
                                    axis=0,
                                ),
                            )
                for tt in range(tg):
                    t = tg0 + tt
                    scr = wpool.tile([P, 2, 2, C], bf16, tag="scr")
                    g_r = g[:].rearrange(
                        "p (tl y x b c) -> p tl y x b c",
                        tl=tg, y=2, x=2, b=BL, c=C,
                    )
                    v_r = v_sb[:, t * 4 * C : (t + 1) * 4 * C].rearrange(
                        "p (y x c) -> p y x c", y=2, x=2, c=C
                    )
                    for bb in range(BL):
                        zcol = z_sb[:, t * BL + bb : t * BL + bb + 1]
                        if compute == "ttr":
                            nc.vector.tensor_tensor_reduce(
                                out=scr[:],
                                in0=g_r[:, tt, :, :, bb, :],
                                in1=v_r,
                                scale=1.0,
                                scalar=0.0,
                                op0=mybir.AluOpType.mult,
                                op1=mybir.AluOpType.add,
                                accum_out=zcol,
                                opt_aps=False,
                            )
                        else:
                            nc.vector.scalar_tensor_tensor(
                                out=scr[:],
                                in0=g_r[:, tt, :, :, bb, :],
                                scalar=1.0,
                                in1=v_r,
                                op0=mybir.AluOpType.mult,
                                op1=mybir.AluOpType.mult,
                                accum_out=zcol,
                            )

            # epilogue: z += bias (broadcast over b)
            ze = cpool.tile([P, nt * BL], f32)
            nc.vector.tensor_tensor(
                out=z_sb[:].rearrange("p (t b) -> p t b", b=BL),
                in0=z_sb[:].rearrange("p (t b) -> p t b", b=BL),
                in1=bias_sb[:].unsqueeze(-1).broadcast_to([P, nt, BL]),
                op=mybir.AluOpType.add,
            )
            # out = exp(min(z,0)) + max(z,0)  == elu(z) + 1
            nc.vector.tensor_scalar_min(ze[:], z_sb[:], 0.0)
            nc.scalar.activation(ze[:], ze[:], mybir.ActivationFunctionType.Exp)
            nc.vector.tensor_scalar_max(z_sb[:], z_sb[:], 0.0)
            nc.vector.tensor_add(z_sb[:], z_sb[:], ze[:])
            nc.sync.dma_start(out[:], z_sb[:])

    # populate .instr bytes for extended-inst InstISA subclasses
    # (dma_gather, tensor_tensor_reduce, load_library); raw Bass skips this
    # Bacc.compile() pass and walrus then fails with "ISA wrong length".
    from concourse.library_overlay import lower_extended_insts
    lower_extended_insts(nc)
    if split_waits:
        _split_multi_waits(nc)
    nc.finalize()
    return nc


def _split_multi_waits(nc):
    """The walrus build in this environment only supports ONE sync-wait slot
    per instruction.  Hoist extra waits onto NoOps inserted just before the
    offending instruction (same engine, so sequencer order enforces them)."""
    import concourse.mybir as mybir
    import bass_rust

    for fn in nc.m.functions:
        for blk in fn.blocks:
            new_insts = []
            for ins in blk.instructions:
                si = getattr(ins, "sync_info", None)
                waits = list(si.on_wait) if si is not None else []
                if len(waits) > 1:
                    for j, w in enumerate(waits[:-1]):
                        nop = mybir.InstNoOp(name=f"{ins.name}-w{j}")
                        nop.engine = ins.engine
                        nop.sync_info = bass_rust.SyncInfo(
                            on_wait=[w], on_update=[]
                        )
                        new_insts.append(nop)
                    ins.sync_info = bass_rust.SyncInfo(
                        on_wait=[waits[-1]], on_update=list(si.on_update)
                    )
                new_insts.append(ins)
            blk.instructions[:] = new_insts


def _host_prep(x, mu, W, b):
    bf16 = ml_dtypes.bfloat16

    # --- per-neuron bilinear indices / weights (shared by all cores) ---
    gx = np.clip(mu[:, 0].astype(np.float64), -1.0, 1.0)
    gy = np.clip(mu[:, 1].astype(np.float64), -1.0, 1.0)
    ix = (gx + 1.0) * (Wd * 0.5) - 0.5
    iy = (gy + 1.0) * (H * 0.5) - 0.5
    x0 = np.floor(ix)
    y0 = np.floor(iy)
    wx1 = (ix - x0).astype(np.float32)
    wy1 = (iy - y0).astype(np.float32)
    wx0 = 1.0 - wx1
    wy0 = 1.0 - wy1
    x0i = np.clip(x0.astype(np.int32), 0, Wd - 2)
    y0i = np.clip(y0.astype(np.int32), 0, H - 2)
    p00 = y0i * Wd + x0i            # row index of (y0, x0); pair covers x0, x0+1
    p01 = p00 + Wd                  # row index of (y1, x0)

    def to_pt(a):  # [N, ...] -> [P, NT, ...] with n = t*128 + p
        return np.ascontiguousarray(
            a.reshape(NT, P, *a.shape[1:]).swapaxes(0, 1)
        )

    idx_np = to_pt(np.stack([p00, p01], axis=-1)).reshape(P, NT * 2)

    # int16 wrapped indices for dma_gather: per gather group of TG tiles,
    # i-order = [t0:p00 x128, t0:p01 x128, t1:p00 x128, ...]; index i lives at
    # [i % 16, i // 16]; replicated across the 8 Q7 core partition groups.
    p00_t = p00.reshape(NT, P)
    p01_t = p01.reshape(NT, P)
    cols = []
    for tg0 in range(0, NT, TG):
        arr = np.concatenate(
            [np.stack([p00_t[t], p01_t[t]]).reshape(-1)
             for t in range(tg0, tg0 + TG)]
        )  # [TG*256] in i-order
        cols.append(arr.reshape(-1, 16).T)  # [16, TG*16]
    idx16_np = np.tile(np.hstack(cols).astype(np.int16), (8, 1))  # [128, NT*16]

    w4_full = np.stack(
        [wx0 * wy0, wx1 * wy0, wx0 * wy1, wx1 * wy1], axis=-1
    ).astype(np.float32)  # [N, 4]
    v_full = (w4_full[:, :, None] * W[:, None, :]).astype(bf16)  # [N, 4, C]
    wv_np = to_pt(v_full).reshape(P, NT * 4 * C)
    biasr_np = to_pt(b.astype(np.float32))  # [P, NT]

    # --- per-core x transpose to pixel-major bf16 ---
    xb = x.astype(bf16).reshape(B, C, NPIX)
    xts = []
    for c in range(NCORES):
        xc = xb[c * BL : (c + 1) * BL]                       # [BL, C, NPIX]
        xt_np = np.ascontiguousarray(xc.transpose(2, 0, 1)).reshape(NPIX, ROW)
        xts.append(xt_np)

    shared = {"wv": wv_np, "idx": idx_np, "idx16": idx16_np, "biasr": biasr_np}
    return [{"xt": xts[c], **shared} for c in range(NCORES)]


def _input_names(nc):
    import concourse.mybir as mybir
    names = set()
    for alloc in nc.m.functions[0].allocations:
        if isinstance(alloc, mybir.MemoryLocationSet) and alloc.kind == "ExternalInput":
            names.add(alloc.memorylocations[0].name)
    return names


def _run(in_maps, trace=False, **kwargs):
    global _PROGRAM
    from concourse import bass_utils

    if _PROGRAM is None:
        _PROGRAM = _build_program()
    want = _input_names(_PROGRAM)
    in_maps = [{k: v for k, v in m.items() if k in want} for m in in_maps]
    rr = bass_utils.run_bass_kernel_spmd(
        _PROGRAM, in_maps, core_ids=list(range(NCORES)), trace=trace, **kwargs
    )
    outs = []
    for c in range(NCORES):
        o = np.asarray(rr.results[c]["out"], dtype=np.float32)  # [P, NT*BL]
        o = o.reshape(P, NT, BL).transpose(2, 1, 0).reshape(BL, N)
        outs.append(o)
    return np.concatenate(outs, axis=0), rr


def kernel(x, mu, W, b):
    in_maps = _host_prep(x, mu, W, b)
    out, _ = _run(in_maps)
    return out



# revision 3
# speedup vs baseline: 3.0403x; 3.0403x over previous
"""Trainium2 Bass kernel for PoissonGaussianReadout.

Computation (per reference):
  out[b, n] = elu( sum_c bilinear_sample(x[b, c], mu[n]) * W[n, c] + bias[n] ) + 1

Sharding: data-parallel over batch B=32 across 8 cores (4 images per core).
Every core processes all N=8192 neurons for its 4 images.

Device strategy per core (v2 — TensorE interpolation, no DMA gather):
  - x is host-transposed to pixel-major, window-partitioned layout
    x_sb[p=128, w=32, (b,c)=1024] bf16: window w = 2 pixel rows (128 px),
    partition = pixel within window.
  - Neurons are host-sorted by y0 and packed into 64 tiles of 128.  For
    each tile and each 2-row window its corner rows touch, a one-hot
    interpolation matrix S[128 px, 128 slots] (bilinear corner weights,
    rows split across windows for odd y0) is host-built.
  - TensorE: f[slot, (b,c)] = sum_w S_w^T @ x_window — PSUM-accumulated
    over the tile's windows (~1.95 avg).  The bilinear gather+lerp is
    thus a dense matmul.
  - ScalarE drains PSUM f32 -> SBUF bf16.
  - DVE: z[slot, b] = sum_c f[slot,b,c] * W[slot,c] via tensor_tensor
    mult (2x bf16 mode) + tensor_reduce over c.
  - Epilogue: out = exp(min(z+bias,0)) + max(z+bias,0)  (== elu(z)+1).
"""

import numpy as np
import ml_dtypes

B, C, H, Wd, N = 32, 256, 64, 64, 8192
NCORES = 8
BL = B // NCORES          # 4 images per core
P = 128                   # partitions / neurons per tile
NT = N // P               # 64 neuron tiles
NW = (H // 2)             # 32 two-row windows of 128 pixels
ROW = BL * C              # 1024 elements per (b,c) pixel row
NPIX = H * Wd             # 4096

_PROGRAM = None
_PREP = None              # (seg_wins, perm) used to build the program


def _build_program(seg_wins):
    """seg_wins: list of per-tile window-index lists (len NT)."""
    import concourse.bass as bass
    import concourse.mybir as mybir
    import concourse.tile as tile

    bf16 = mybir.dt.bfloat16
    f32 = mybir.dt.float32

    nseg = sum(len(w) for w in seg_wins)

    nc = bass.Bass("TRN2")

    xt = nc.dram_tensor("xt", [P, NW * ROW], bf16, kind="ExternalInput")
    ss = nc.dram_tensor("ss", [P, nseg * P], bf16, kind="ExternalInput")
    ws = nc.dram_tensor("ws", [P, NT * C], bf16, kind="ExternalInput")
    biasr = nc.dram_tensor("biasr", [P, NT], f32, kind="ExternalInput")
    out = nc.dram_tensor("out", [P, NT * BL], f32, kind="ExternalOutput")

    # per-tile starting segment index
    seg0 = np.cumsum([0] + [len(w) for w in seg_wins])
    TG = 8  # tiles per S/W DMA chunk

    with tile.TileContext(nc) as tc:
        with (
            tc.tile_pool(name="const", bufs=1) as cpool,
            tc.tile_pool(name="fpool", bufs=3) as fpool,
            tc.tile_pool(name="upool", bufs=3) as upool,
            tc.tile_pool(name="psum", bufs=4, space="PSUM") as ppool,
        ):
            x_sb = cpool.tile([P, NW, ROW], bf16)
            s_sb = cpool.tile([P, nseg * P], bf16)
            w_sb = cpool.tile([P, NT, C], bf16)
            bias_sb = cpool.tile([P, NT], f32)
            z_sb = cpool.tile([P, NT, BL], f32)

            # interleave DMAs so tile-t dependencies land early:
            # group g covers tiles [8g, 8g+8) -> x windows, S segs, W rows.
            nc.sync.dma_start(bias_sb[:], biasr[:])
            wdone = 0
            for g in range(NT // TG):
                t0, t1 = g * TG, (g + 1) * TG
                wneed = max(w for t in range(t0, t1) for w in seg_wins[t]) + 1
                for w in range(wdone, wneed):
                    nc.sync.dma_start(
                        x_sb[:, w], xt[:, w * ROW : (w + 1) * ROW]
                    )
                wdone = wneed
                s0, s1 = seg0[t0] * P, seg0[t1] * P
                nc.sync.dma_start(s_sb[:, s0:s1], ss[:, s0:s1])
                nc.sync.dma_start(
                    w_sb[:, t0:t1].rearrange("p t c -> p (t c)"),
                    ws[:, t0 * C : t1 * C],
                )
            for w in range(wdone, NW):
                nc.sync.dma_start(x_sb[:, w], xt[:, w * ROW : (w + 1) * ROW])

            for t in range(NT):
                wins = seg_wins[t]
                f_ps = ppool.tile([P, 2, 512], f32)
                for h in range(2):
                    for i, w in enumerate(wins):
                        s = seg0[t] + i
                        nc.tensor.matmul(
                            f_ps[:, h],
                            s_sb[:, s * P : (s + 1) * P],
                            x_sb[:, w, h * 512 : (h + 1) * 512],
                            start=(i == 0),
                            stop=(i == len(wins) - 1),
                        )
                f_bf = fpool.tile([P, BL, C], bf16, tag="f")
                nc.scalar.copy(
                    f_bf[:].rearrange("p b c -> p (b c)"),
                    f_ps[:].rearrange("p a k -> p (a k)"),
                )
                u = upool.tile([P, BL, C], bf16, tag="u")
                nc.vector.tensor_tensor(
                    out=u[:],
                    in0=f_bf[:],
                    in1=w_sb[:, t : t + 1, :].broadcast_to([P, BL, C]),
                    op=mybir.AluOpType.mult,
                )
                nc.vector.tensor_reduce(
                    out=z_sb[:, t],
                    in_=u[:],
                    axis=mybir.AxisListType.X,
                    op=mybir.AluOpType.add,
                )

            # epilogue: z += bias (broadcast over b); out = elu(z) + 1
            ze = cpool.tile([P, NT * BL], f32)
            zf = z_sb[:].rearrange("p t b -> p (t b)")
            nc.vector.tensor_tensor(
                out=z_sb[:],
                in0=z_sb[:],
                in1=bias_sb[:].unsqueeze(-1).broadcast_to([P, NT, BL]),
                op=mybir.AluOpType.add,
            )
            nc.vector.tensor_scalar_min(ze[:], zf, 0.0)
            nc.scalar.activation(ze[:], ze[:], mybir.ActivationFunctionType.Exp)
            nc.vector.tensor_scalar_max(zf, zf, 0.0)
            nc.vector.tensor_add(zf, zf, ze[:])
            nc.sync.dma_start(out[:], zf)

    from concourse.library_overlay import lower_extended_insts

    lower_extended_insts(nc)
    _split_multi_waits(nc)
    nc.finalize()
    return nc


def _split_multi_waits(nc):
    """The walrus build in this environment only supports ONE sync-wait slot
    per instruction.  Hoist extra waits onto NoOps inserted just before the
    offending instruction (same engine, so sequencer order enforces them)."""
    import concourse.mybir as mybir
    import bass_rust

    for fn in nc.m.functions:
        for blk in fn.blocks:
            new_insts = []
            for ins in blk.instructions:
                si = getattr(ins, "sync_info", None)
                waits = list(si.on_wait) if si is not None else []
                if len(waits) > 1:
                    for j, w in enumerate(waits[:-1]):
                        nop = mybir.InstNoOp(name=f"{ins.name}-w{j}")
                        nop.engine = ins.engine
                        nop.sync_info = bass_rust.SyncInfo(
                            on_wait=[w], on_update=[]
                        )
                        new_insts.append(nop)
                    ins.sync_info = bass_rust.SyncInfo(
                        on_wait=[waits[-1]], on_update=list(si.on_update)
                    )
                new_insts.append(ins)
            blk.instructions[:] = new_insts


def _host_prep(x, mu, W, b):
    bf16 = ml_dtypes.bfloat16

    # --- per-neuron bilinear indices / weights ---
    gx = np.clip(mu[:, 0].astype(np.float64), -1.0, 1.0)
    gy = np.clip(mu[:, 1].astype(np.float64), -1.0, 1.0)
    ix = (gx + 1.0) * (Wd * 0.5) - 0.5
    iy = (gy + 1.0) * (H * 0.5) - 0.5
    x0 = np.floor(ix)
    y0 = np.floor(iy)
    wx1 = (ix - x0).astype(np.float32)
    wy1 = (iy - y0).astype(np.float32)
    wx0 = 1.0 - wx1
    wy0 = 1.0 - wy1
    x0i = np.clip(x0.astype(np.int32), 0, Wd - 2)
    y0i = np.clip(y0.astype(np.int32), 0, H - 2)

    # sort neurons by y0 -> tiles of 128 spanning ~2 windows each
    perm = np.argsort(y0i, kind="stable")
    y0s, x0s = y0i[perm], x0i[perm]
    wgt = np.stack(
        [wx0 * wy0, wx1 * wy0, wx0 * wy1, wx1 * wy1], axis=-1
    ).astype(np.float32)[perm]  # [N, 4] corner weights (00,10,01,11)

    # segment structure + S matrices
    seg_wins = []
    s_blocks = []
    for t in range(NT):
        sl = slice(t * P, (t + 1) * P)
        yy, xx, wg = y0s[sl], x0s[sl], wgt[sl]
        wins = sorted(set(yy // 2) | set((yy + 1) // 2))
        seg_wins.append(list(wins))
        for w in wins:
            S = np.zeros((P, P), dtype=np.float32)
            j = np.arange(P)
            for r, w0, w1 in ((yy, wg[:, 0], wg[:, 1]), (yy + 1, wg[:, 2], wg[:, 3])):
                m = (r // 2) == w
                p = 64 * (r - 2 * w) + xx
                S[p[m], j[m]] += w0[m]
                S[p[m] + 1, j[m]] += w1[m]
            s_blocks.append(S)
    ss_np = np.concatenate(s_blocks, axis=1).astype(bf16)  # [P, nseg*P]

    ws_np = np.ascontiguousarray(
        W[perm].astype(bf16).reshape(NT, P, C).swapaxes(0, 1).reshape(P, NT * C)
    )
    biasr_np = np.ascontiguousarray(
        b[perm].astype(np.float32).reshape(NT, P).T
    )

    # --- per-core x in window-partitioned pixel-major layout ---
    # xt[p, w*ROW + bl*C + c] = x[b0+bl, c, pix = w*128 + p]
    xb = x.astype(bf16).reshape(B, C, NPIX)
    xts = []
    for cix in range(NCORES):
        xc = xb[cix * BL : (cix + 1) * BL]          # [BL, C, NPIX]
        # -> [NPIX, BL, C] -> [NW, 128, ROW] -> [128, NW, ROW]
        xt_np = np.ascontiguousarray(
            xc.transpose(2, 0, 1)
            .reshape(NW, P, ROW)
            .swapaxes(0, 1)
            .reshape(P, NW * ROW)
        )
        xts.append(xt_np)

    shared = {"ss": ss_np, "ws": ws_np, "biasr": biasr_np}
    in_maps = [{"xt": xts[cix], **shared} for cix in range(NCORES)]
    return in_maps, seg_wins, perm


def _run(prep, trace=False, **kwargs):
    global _PROGRAM, _PREP
    from concourse import bass_utils

    in_maps, seg_wins, perm = prep
    if _PROGRAM is None:
        _PROGRAM = _build_program(seg_wins)
        _PREP = seg_wins
    rr = bass_utils.run_bass_kernel_spmd(
        _PROGRAM, in_maps, core_ids=list(range(NCORES)), trace=trace, **kwargs
    )
    inv = np.empty(N, dtype=np.int64)
    inv[perm] = np.arange(N)
    outs = []
    for cix in range(NCORES):
        o = np.asarray(rr.results[cix]["out"], dtype=np.float32)  # [P, NT*BL]
        o = o.reshape(P, NT, BL).transpose(2, 1, 0).reshape(BL, N)  # sorted order
        outs.append(o[:, inv])
    return np.concatenate(outs, axis=0), rr


def kernel(x, mu, W, b):
    prep = _host_prep(x, mu, W, b)
    out, _ = _run(prep)
    return out


# revision 4
# speedup vs baseline: 3.1400x; 1.0328x over previous
"""Trainium2 Bass kernel for PoissonGaussianReadout.

Computation (per reference):
  out[b, n] = elu( sum_c bilinear_sample(x[b, c], mu[n]) * W[n, c] + bias[n] ) + 1

Sharding: data-parallel over batch B=32 across 8 cores (4 images per core).
Every core processes all N=8192 neurons for its 4 images.

Device strategy per core (v3 — TensorE interpolation, no DMA gather):
  - x is host-transposed to pixel-major, window-partitioned layout
    x_sb[p=128, w=32, (b,c)=1024] bf16: window w = 2 pixel rows (128 px),
    partition = pixel within window.
  - Neurons are host-sorted by y0 and packed into 64 tiles of 128.  For
    each tile and each 2-row window its corner rows touch, a one-hot
    interpolation matrix S[128 px, 128 slots] (bilinear corner weights,
    rows split across windows for odd y0) is host-built.
  - TensorE: f[slot, (b,c)] = sum_w S_w^T @ x_window — PSUM-accumulated
    over the tile's windows (~1.95 avg).  The bilinear gather+lerp is
    thus a dense matmul.
  - ScalarE drains PSUM f32 -> SBUF bf16 (two tiles per activation).
  - DVE: z[slot, b] = sum_c f[slot,b,c] * W[slot,c] via tensor_tensor
    mult (2x bf16) + tensor_reduce over c (bf16 out -> 4x mode; the DVE
    reduction accumulator is fp32 internally, only the final z rounds).
  - Epilogue: out = exp(min(z+bias,0)) + max(z+bias,0)  (== elu(z)+1).
"""

import numpy as np
import ml_dtypes

B, C, H, Wd, N = 32, 256, 64, 64, 8192
NCORES = 8
BL = B // NCORES          # 4 images per core
P = 128                   # partitions / neurons per tile
NT = N // P               # 64 neuron tiles
NW = (H // 2)             # 32 two-row windows of 128 pixels
ROW = BL * C              # 1024 elements per (b,c) pixel row
NPIX = H * Wd             # 4096

_PROGRAM = None


def _build_program(seg_wins):
    """seg_wins: list of per-tile window-index lists (len NT)."""
    import concourse.bass as bass
    import concourse.mybir as mybir
    import concourse.tile as tile

    bf16 = mybir.dt.bfloat16
    f32 = mybir.dt.float32

    nseg = sum(len(w) for w in seg_wins)

    nc = bass.Bass("TRN2")

    xt = nc.dram_tensor("xt", [P, NW * ROW], bf16, kind="ExternalInput")
    ss = nc.dram_tensor("ss", [P, nseg * P], bf16, kind="ExternalInput")
    ws = nc.dram_tensor("ws", [P, NT * C], bf16, kind="ExternalInput")
    biasr = nc.dram_tensor("biasr", [P, NT], f32, kind="ExternalInput")
    out = nc.dram_tensor("out", [P, NT * BL], f32, kind="ExternalOutput")

    # per-tile starting segment index
    seg0 = np.cumsum([0] + [len(w) for w in seg_wins])
    TG = 8  # tiles per S/W DMA chunk

    with tile.TileContext(nc) as tc:
        with (
            tc.tile_pool(name="const", bufs=1) as cpool,
            tc.tile_pool(name="fpool", bufs=3) as fpool,
            tc.tile_pool(name="upool", bufs=3) as upool,
            tc.tile_pool(name="psum", bufs=2, space="PSUM") as ppool,
        ):
            x_sb = cpool.tile([P, NW, ROW], bf16)
            s_sb = cpool.tile([P, nseg * P], bf16)
            w_sb = cpool.tile([P, NT, C], bf16)
            bias_sb = cpool.tile([P, NT], f32)
            z_sb = cpool.tile([P, NT, BL], bf16)

            # interleave DMAs so tile-t dependencies land early:
            # group g covers tiles [8g, 8g+8) -> S segs, W rows, x windows.
            wdone = 0
            for g in range(NT // TG):
                t0, t1 = g * TG, (g + 1) * TG
                s0, s1 = seg0[t0] * P, seg0[t1] * P
                nc.sync.dma_start(s_sb[:, s0:s1], ss[:, s0:s1])
                nc.sync.dma_start(
                    w_sb[:, t0:t1].rearrange("p t c -> p (t c)"),
                    ws[:, t0 * C : t1 * C],
                )
                wneed = max(w for t in range(t0, t1) for w in seg_wins[t]) + 1
                for w in range(wdone, wneed):
                    nc.sync.dma_start(
                        x_sb[:, w], xt[:, w * ROW : (w + 1) * ROW]
                    )
                wdone = wneed
            for w in range(wdone, NW):
                nc.sync.dma_start(x_sb[:, w], xt[:, w * ROW : (w + 1) * ROW])
            nc.sync.dma_start(bias_sb[:], biasr[:])

            for q in range(NT // 2):  # tile pairs
                f_ps = ppool.tile([P, 2, 2, 512], f32)
                for tp in range(2):
                    t = 2 * q + tp
                    wins = seg_wins[t]
                    for i, w in enumerate(wins):
                        s = seg0[t] + i
                        for h in range(2):
                            nc.tensor.matmul(
                                f_ps[:, tp, h],
                                s_sb[:, s * P : (s + 1) * P],
                                x_sb[:, w, h * 512 : (h + 1) * 512],
                                start=(i == 0),
                                stop=(i == len(wins) - 1),
                            )
                f_bf = fpool.tile([P, 2, BL, C], bf16, tag="f")
                nc.scalar.copy(
                    f_bf[:].rearrange("p t b c -> p (t b c)"),
                    f_ps[:].rearrange("p t a k -> p (t a k)"),
                )
                u = upool.tile([P, 2, BL, C], bf16, tag="u")
                nc.vector.tensor_tensor(
                    out=u[:],
                    in0=f_bf[:],
                    in1=w_sb[:, 2 * q : 2 * q + 2, :]
                    .unsqueeze(2)
                    .broadcast_to([P, 2, BL, C]),
                    op=mybir.AluOpType.mult,
                )
                with nc.allow_low_precision(
                    "z rounds to bf16 only on the final write; DVE reduce "
                    "accumulator is fp32 internally"
                ):
                    nc.vector.tensor_reduce(
                        out=z_sb[:, 2 * q : 2 * q + 2],
                        in_=u[:],
                        axis=mybir.AxisListType.X,
                        op=mybir.AluOpType.add,
                    )

            # epilogue: z += bias (broadcast over b); out = elu(z) + 1
            zf = cpool.tile([P, NT * BL], f32)
            ze = cpool.tile([P, NT * BL], f32)
            nc.vector.tensor_tensor(
                out=zf[:].rearrange("p (t b) -> p t b", b=BL),
                in0=z_sb[:],
                in1=bias_sb[:].unsqueeze(-1).broadcast_to([P, NT, BL]),
                op=mybir.AluOpType.add,
            )
            nc.vector.tensor_scalar_min(ze[:], zf[:], 0.0)
            nc.scalar.activation(ze[:], ze[:], mybir.ActivationFunctionType.Exp)
            nc.vector.tensor_scalar_max(zf[:], zf[:], 0.0)
            nc.vector.tensor_add(zf[:], zf[:], ze[:])
            nc.sync.dma_start(out[:], zf[:])

    from concourse.library_overlay import lower_extended_insts

    lower_extended_insts(nc)
    _split_multi_waits(nc)
    nc.finalize()
    return nc


def _split_multi_waits(nc):
    """The walrus build in this environment only supports ONE sync-wait slot
    per instruction.  Hoist extra waits onto NoOps inserted just before the
    offending instruction (same engine, so sequencer order enforces them)."""
    import concourse.mybir as mybir
    import bass_rust

    for fn in nc.m.functions:
        for blk in fn.blocks:
            new_insts = []
            for ins in blk.instructions:
                si = getattr(ins, "sync_info", None)
                waits = list(si.on_wait) if si is not None else []
                if len(waits) > 1:
                    for j, w in enumerate(waits[:-1]):
                        nop = mybir.InstNoOp(name=f"{ins.name}-w{j}")
                        nop.engine = ins.engine
                        nop.sync_info = bass_rust.SyncInfo(
                            on_wait=[w], on_update=[]
                        )
                        new_insts.append(nop)
                    ins.sync_info = bass_rust.SyncInfo(
                        on_wait=[waits[-1]], on_update=list(si.on_update)
                    )
                new_insts.append(ins)
            blk.instructions[:] = new_insts


def _host_prep(x, mu, W, b):
    bf16 = ml_dtypes.bfloat16

    # --- per-neuron bilinear indices / weights ---
    gx = np.clip(mu[:, 0].astype(np.float64), -1.0, 1.0)
    gy = np.clip(mu[:, 1].astype(np.float64), -1.0, 1.0)
    ix = (gx + 1.0) * (Wd * 0.5) - 0.5
    iy = (gy + 1.0) * (H * 0.5) - 0.5
    x0 = np.floor(ix)
    y0 = np.floor(iy)
    wx1 = (ix - x0).astype(np.float32)
    wy1 = (iy - y0).astype(np.float32)
    wx0 = 1.0 - wx1
    wy0 = 1.0 - wy1
    x0i = np.clip(x0.astype(np.int32), 0, Wd - 2)
    y0i = np.clip(y0.astype(np.int32), 0, H - 2)

    # sort neurons by y0 -> tiles of 128 spanning ~2 windows each
    perm = np.argsort(y0i, kind="stable")
    y0s, x0s = y0i[perm], x0i[perm]
    wgt = np.stack(
        [wx0 * wy0, wx1 * wy0, wx0 * wy1, wx1 * wy1], axis=-1
    ).astype(np.float32)[perm]  # [N, 4] corner weights (00,10,01,11)

    # segment structure + S matrices
    seg_wins = []
    s_blocks = []
    for t in range(NT):
        sl = slice(t * P, (t + 1) * P)
        yy, xx, wg = y0s[sl], x0s[sl], wgt[sl]
        wins = sorted(set(yy // 2) | set((yy + 1) // 2))
        seg_wins.append(list(wins))
        for w in wins:
            S = np.zeros((P, P), dtype=np.float32)
            j = np.arange(P)
            for r, w0, w1 in ((yy, wg[:, 0], wg[:, 1]), (yy + 1, wg[:, 2], wg[:, 3])):
                m = (r // 2) == w
                p = 64 * (r - 2 * w) + xx
                S[p[m], j[m]] += w0[m]
                S[p[m] + 1, j[m]] += w1[m]
            s_blocks.append(S)
    ss_np = np.concatenate(s_blocks, axis=1).astype(bf16)  # [P, nseg*P]

    ws_np = np.ascontiguousarray(
        W[perm].astype(bf16).reshape(NT, P, C).swapaxes(0, 1).reshape(P, NT * C)
    )
    biasr_np = np.ascontiguousarray(
        b[perm].astype(np.float32).reshape(NT, P).T
    )

    # --- per-core x in window-partitioned pixel-major layout ---
    # xt[p, w*ROW + bl*C + c] = x[b0+bl, c, pix = w*128 + p]
    xb = x.astype(bf16).reshape(B, C, NPIX)
    xts = []
    for cix in range(NCORES):
        xc = xb[cix * BL : (cix + 1) * BL]          # [BL, C, NPIX]
        xt_np = np.ascontiguousarray(
            xc.transpose(2, 0, 1)
            .reshape(NW, P, ROW)
            .swapaxes(0, 1)
            .reshape(P, NW * ROW)
        )
        xts.append(xt_np)

    shared = {"ss": ss_np, "ws": ws_np, "biasr": biasr_np}
    in_maps = [{"xt": xts[cix], **shared} for cix in range(NCORES)]
    return in_maps, seg_wins, perm


def _run(prep, trace=False, **kwargs):
    global _PROGRAM
    from concourse import bass_utils

    in_maps, seg_wins, perm = prep
    if _PROGRAM is None:
        _PROGRAM = _build_program(seg_wins)
    rr = bass_utils.run_bass_kernel_spmd(
        _PROGRAM, in_maps, core_ids=list(range(NCORES)), trace=trace, **kwargs
    )
    inv = np.empty(N, dtype=np.int64)
    inv[perm] = np.arange(N)
    outs = []
    for cix in range(NCORES):
        o = np.asarray(rr.results[cix]["out"], dtype=np.float32)  # [P, NT*BL]
        o = o.reshape(P, NT, BL).transpose(2, 1, 0).reshape(BL, N)  # sorted order
        outs.append(o[:, inv])
    return np.concatenate(outs, axis=0), rr


def kernel(x, mu, W, b):
    prep = _host_prep(x, mu, W, b)
    out, _ = _run(prep)
    return out


# revision 6
# speedup vs baseline: 3.6172x; 1.1520x over previous
"""Trainium2 Bass kernel for PoissonGaussianReadout.

Computation (per reference):
  out[b, n] = elu( sum_c bilinear_sample(x[b, c], mu[n]) * W[n, c] + bias[n] ) + 1

Sharding: data-parallel over batch B=32 across 8 cores (4 images per core).
Every core processes all N=8192 neurons for its 4 images.

Device strategy per core (v3 — TensorE interpolation, no DMA gather):
  - x is host-transposed to pixel-major, window-partitioned layout
    x_sb[p=128, w=32, (b,c)=1024] bf16: window w = 2 pixel rows (128 px),
    partition = pixel within window.
  - Neurons are host-sorted by y0 and packed into 64 tiles of 128.  For
    each tile and each 2-row window its corner rows touch, a one-hot
    interpolation matrix S[128 px, 128 slots] (bilinear corner weights,
    rows split across windows for odd y0) is host-built.
  - TensorE: f[slot, (b,c)] = sum_w S_w^T @ x_window — PSUM-accumulated
    over the tile's windows (~1.95 avg).  The bilinear gather+lerp is
    thus a dense matmul.
  - ScalarE drains PSUM f32 -> SBUF bf16 (two tiles per activation).
  - DVE: z[slot, b] = sum_c f[slot,b,c] * W[slot,c] via tensor_tensor
    mult (2x bf16) + tensor_reduce over c (bf16 out -> 4x mode; the DVE
    reduction accumulator is fp32 internally, only the final z rounds).
  - Epilogue: out = exp(min(z+bias,0)) + max(z+bias,0)  (== elu(z)+1).
"""

import numpy as np
import ml_dtypes

B, C, H, Wd, N = 32, 256, 64, 64, 8192
NCORES = 8
BL = B // NCORES          # 4 images per core
P = 128                   # partitions / neurons per tile
NT = N // P               # 64 neuron tiles
NW = (H // 2)             # 32 two-row windows of 128 pixels
ROW = BL * C              # 1024 elements per (b,c) pixel row
NPIX = H * Wd             # 4096

_PROGRAM = None


def _build_program(seg_wins):
    """seg_wins: list of per-tile window-index lists (len NT)."""
    import concourse.bass as bass
    import concourse.mybir as mybir
    import concourse.tile as tile

    bf16 = mybir.dt.bfloat16
    f32 = mybir.dt.float32

    nseg = sum(len(w) for w in seg_wins)

    nc = bass.Bass("TRN2")

    xt = nc.dram_tensor("xt", [P, NW * ROW], bf16, kind="ExternalInput")
    ss = nc.dram_tensor("ss", [P, nseg * P], bf16, kind="ExternalInput")
    ws = nc.dram_tensor("ws", [P, NT * C], bf16, kind="ExternalInput")
    biasr = nc.dram_tensor("biasr", [P, NT], f32, kind="ExternalInput")
    out = nc.dram_tensor("out", [P, NT * BL], f32, kind="ExternalOutput")

    # per-tile starting segment index
    seg0 = np.cumsum([0] + [len(w) for w in seg_wins])
    TG = 8  # tiles per S/W DMA chunk

    with tile.TileContext(nc) as tc:
        with (
            tc.tile_pool(name="const", bufs=1) as cpool,
            tc.tile_pool(name="fpool", bufs=3) as fpool,
            tc.tile_pool(name="upool", bufs=3) as upool,
            tc.tile_pool(name="psum", bufs=2, space="PSUM") as ppool,
        ):
            x_sb = cpool.tile([P, NW, ROW], bf16)
            s_sb = cpool.tile([P, nseg * P], bf16)
            w_sb = cpool.tile([P, NT, C], bf16)
            bias_sb = cpool.tile([P, NT], f32)
            z_sb = cpool.tile([P, NT, BL], f32)

            # interleave DMAs so tile-t dependencies land early:
            # group g covers tiles [8g, 8g+8) -> S segs, W rows, x windows.
            wdone = 0
            for g in range(NT // TG):
                t0, t1 = g * TG, (g + 1) * TG
                s0, s1 = seg0[t0] * P, seg0[t1] * P
                nc.sync.dma_start(s_sb[:, s0:s1], ss[:, s0:s1])
                nc.sync.dma_start(
                    w_sb[:, t0:t1].rearrange("p t c -> p (t c)"),
                    ws[:, t0 * C : t1 * C],
                )
                wneed = max(w for t in range(t0, t1) for w in seg_wins[t]) + 1
                for w in range(wdone, wneed):
                    nc.sync.dma_start(
                        x_sb[:, w], xt[:, w * ROW : (w + 1) * ROW]
                    )
                wdone = wneed
            for w in range(wdone, NW):
                nc.sync.dma_start(x_sb[:, w], xt[:, w * ROW : (w + 1) * ROW])
            nc.sync.dma_start(bias_sb[:], biasr[:])

            for q in range(NT // 2):  # tile pairs
                f_ps = ppool.tile([P, 2, 2, 512], f32)
                for tp in range(2):
                    t = 2 * q + tp
                    wins = seg_wins[t]
                    for i, w in enumerate(wins):
                        s = seg0[t] + i
                        for h in range(2):
                            nc.tensor.matmul(
                                f_ps[:, tp, h],
                                s_sb[:, s * P : (s + 1) * P],
                                x_sb[:, w, h * 512 : (h + 1) * 512],
                                start=(i == 0),
                                stop=(i == len(wins) - 1),
                            )
                f_bf = fpool.tile([P, 2, BL, C], bf16, tag="f")
                nc.scalar.copy(
                    f_bf[:].rearrange("p t b c -> p (t b c)"),
                    f_ps[:].rearrange("p t a k -> p (t a k)"),
                )
                u = upool.tile([P, 2, BL, C], bf16, tag="u")
                for tp in range(2):
                    t = 2 * q + tp
                    for bb in range(BL):
                        nc.vector.scalar_tensor_tensor(
                            out=u[:, tp, bb],
                            in0=f_bf[:, tp, bb],
                            scalar=1.0,
                            in1=w_sb[:, t, :],
                            op0=mybir.AluOpType.mult,
                            op1=mybir.AluOpType.mult,
                            accum_out=z_sb[:, t, bb : bb + 1],
                        )

            # epilogue: z += bias (broadcast over b); out = elu(z) + 1
            zf = cpool.tile([P, NT * BL], f32)
            ze = cpool.tile([P, NT * BL], f32)
            nc.vector.tensor_tensor(
                out=zf[:].rearrange("p (t b) -> p t b", b=BL),
                in0=z_sb[:],
                in1=bias_sb[:].unsqueeze(-1).broadcast_to([P, NT, BL]),
                op=mybir.AluOpType.add,
            )
            nc.vector.tensor_scalar_min(ze[:], zf[:], 0.0)
            nc.scalar.activation(ze[:], ze[:], mybir.ActivationFunctionType.Exp)
            nc.vector.tensor_scalar_max(zf[:], zf[:], 0.0)
            nc.vector.tensor_add(zf[:], zf[:], ze[:])
            nc.sync.dma_start(out[:], zf[:])

    from concourse.library_overlay import lower_extended_insts

    lower_extended_insts(nc)
    _split_multi_waits(nc)
    nc.finalize()
    return nc


def _split_multi_waits(nc):
    """The walrus build in this environment only supports ONE sync-wait slot
    per instruction.  Hoist extra waits onto NoOps inserted just before the
    offending instruction (same engine, so sequencer order enforces them)."""
    import concourse.mybir as mybir
    import bass_rust

    for fn in nc.m.functions:
        for blk in fn.blocks:
            new_insts = []
            for ins in blk.instructions:
                si = getattr(ins, "sync_info", None)
                waits = list(si.on_wait) if si is not None else []
                if len(waits) > 1:
                    for j, w in enumerate(waits[:-1]):
                        nop = mybir.InstNoOp(name=f"{ins.name}-w{j}")
                        nop.engine = ins.engine
                        nop.sync_info = bass_rust.SyncInfo(
                            on_wait=[w], on_update=[]
                        )
                        new_insts.append(nop)
                    ins.sync_info = bass_rust.SyncInfo(
                        on_wait=[waits[-1]], on_update=list(si.on_update)
                    )
                new_insts.append(ins)
            blk.instructions[:] = new_insts


def _host_prep(x, mu, W, b):
    bf16 = ml_dtypes.bfloat16

    # --- per-neuron bilinear indices / weights ---
    gx = np.clip(mu[:, 0].astype(np.float64), -1.0, 1.0)
    gy = np.clip(mu[:, 1].astype(np.float64), -1.0, 1.0)
    ix = (gx + 1.0) * (Wd * 0.5) - 0.5
    iy = (gy + 1.0) * (H * 0.5) - 0.5
    x0 = np.floor(ix)
    y0 = np.floor(iy)
    wx1 = (ix - x0).astype(np.float32)
    wy1 = (iy - y0).astype(np.float32)
    wx0 = 1.0 - wx1
    wy0 = 1.0 - wy1
    x0i = np.clip(x0.astype(np.int32), 0, Wd - 2)
    y0i = np.clip(y0.astype(np.int32), 0, H - 2)

    # sort neurons by y0 -> tiles of 128 spanning ~2 windows each
    perm = np.argsort(y0i, kind="stable")
    y0s, x0s = y0i[perm], x0i[perm]
    wgt = np.stack(
        [wx0 * wy0, wx1 * wy0, wx0 * wy1, wx1 * wy1], axis=-1
    ).astype(np.float32)[perm]  # [N, 4] corner weights (00,10,01,11)

    # segment structure + S matrices
    seg_wins = []
    s_blocks = []
    for t in range(NT):
        sl = slice(t * P, (t + 1) * P)
        yy, xx, wg = y0s[sl], x0s[sl], wgt[sl]
        wins = sorted(set(yy // 2) | set((yy + 1) // 2))
        seg_wins.append(list(wins))
        for w in wins:
            S = np.zeros((P, P), dtype=np.float32)
            j = np.arange(P)
            for r, w0, w1 in ((yy, wg[:, 0], wg[:, 1]), (yy + 1, wg[:, 2], wg[:, 3])):
                m = (r // 2) == w
                p = 64 * (r - 2 * w) + xx
                S[p[m], j[m]] += w0[m]
                S[p[m] + 1, j[m]] += w1[m]
            s_blocks.append(S)
    ss_np = np.concatenate(s_blocks, axis=1).astype(bf16)  # [P, nseg*P]

    ws_np = np.ascontiguousarray(
        W[perm].astype(bf16).reshape(NT, P, C).swapaxes(0, 1).reshape(P, NT * C)
    )
    biasr_np = np.ascontiguousarray(
        b[perm].astype(np.float32).reshape(NT, P).T
    )

    # --- per-core x in window-partitioned pixel-major layout ---
    # xt[p, w*ROW + bl*C + c] = x[b0+bl, c, pix = w*128 + p]
    xb = x.astype(bf16).reshape(B, C, NPIX)
    xts = []
    for cix in range(NCORES):
        xc = xb[cix * BL : (cix + 1) * BL]          # [BL, C, NPIX]
        xt_np = np.ascontiguousarray(
            xc.transpose(2, 0, 1)
            .reshape(NW, P, ROW)
            .swapaxes(0, 1)
            .reshape(P, NW * ROW)
        )
        xts.append(xt_np)

    shared = {"ss": ss_np, "ws": ws_np, "biasr": biasr_np}
    in_maps = [{"xt": xts[cix], **shared} for cix in range(NCORES)]
    return in_maps, seg_wins, perm


def _run(prep, trace=False, **kwargs):
    global _PROGRAM
    from concourse import bass_utils

    in_maps, seg_wins, perm = prep
    if _PROGRAM is None:
        _PROGRAM = _build_program(seg_wins)
    rr = bass_utils.run_bass_kernel_spmd(
        _PROGRAM, in_maps, core_ids=list(range(NCORES)), trace=trace, **kwargs
    )
    inv = np.empty(N, dtype=np.int64)
    inv[perm] = np.arange(N)
    outs = []
    for cix in range(NCORES):
        o = np.asarray(rr.results[cix]["out"], dtype=np.float32)  # [P, NT*BL]
        o = o.reshape(P, NT, BL).transpose(2, 1, 0).reshape(BL, N)  # sorted order
        outs.append(o[:, inv])
    return np.concatenate(outs, axis=0), rr


def kernel(x, mu, W, b):
    prep = _host_prep(x, mu, W, b)
    out, _ = _run(prep)
    return out


# revision 11
# speedup vs baseline: 4.6791x; 1.2936x over previous
"""Trainium2 Bass kernel for PoissonGaussianReadout.

Computation (per reference):
  out[b, n] = elu( sum_c bilinear_sample(x[b, c], mu[n]) * W[n, c] + bias[n] ) + 1

Sharding: data-parallel over batch B=32 across 8 cores (4 images per core).
Every core processes all N=8192 neurons for its 4 images.

Device strategy per core (v4 — "rect-B": channel dot on TensorE first,
then bilinear interpolation as a small DVE weighted sum):
  - Neurons are host-sorted by 8x8 spatial block of their receptive-field
    corner (y0//8, x0//8), packed into 64 tiles of 128.  A tile's corner
    pixels then span 1-2 small rectangles (~9x18 px, FD~152 avg).
  - x is host-transposed channel-major: x[c_part, pixblock, c_chunk, b, pix].
  - TensorE: Y[slot, b, rect-px] = sum_c W[slot, c] * x[c, b, rect-px] —
    stationary = per-tile W chunk [128c x 128n], moving = the rect pixels,
    PSUM-accumulated over the 2 c-chunks.  Per-b-pair matmuls keep each
    output inside one PSUM bank.
  - ScalarE drains PSUM f32 -> SBUF bf16 per bin (rects first-fit packed
    into <=256-col PSUM half-bank bins).
  - DVE: z[slot, b] = sum_px Y[slot, b, px] * S'[slot, px] where S' holds
    the 4 bilinear corner weights (zero elsewhere) — tensor_tensor mult
    (2x bf16) + tensor_reduce, per tile pair.
  - Epilogue: out = exp(min(z+bias,0)) + max(z+bias,0)  (== elu(z)+1).
"""

import numpy as np
import ml_dtypes

B, C, H, Wd, N = 32, 256, 64, 64, 8192
NCORES = 8
BL = B // NCORES          # 4 images per core
P = 128                   # partitions / neurons per tile
NT = N // P               # 64 neuron tiles
NPB = 8                   # pixel blocks (block-rows), 512 px each
NPIX = H * Wd             # 4096
PBSZ = 512                # pixels per block-row

_PROGRAM = None


def _build_program(meta):
    """meta: dict with per-tile rect/bin structure (see _host_prep)."""
    import concourse.bass as bass
    import concourse.mybir as mybir
    import concourse.tile as tile

    bf16 = mybir.dt.bfloat16
    f32 = mybir.dt.float32

    tiles = meta["tiles"]        # per tile: list of bins; bin = list of rects
                                 # rect = (br, rmin, nr, xmin, xl, coloff)
    fdt = meta["fdt"]            # per tile FD (cols)
    fdq = meta["fdq"]            # per pair padded FD
    soff = meta["soff"]          # per pair S' offset (elements per partition)
    ssz = meta["ssz"]
    fdmax = max(fdq)

    nc = bass.Bass("TRN2")

    xt = nc.dram_tensor("xt", [P, NPB * 2 * BL * PBSZ], bf16, kind="ExternalInput")
    ws = nc.dram_tensor("ws", [P, NT * 2 * P], bf16, kind="ExternalInput")
    ss = nc.dram_tensor("ss", [P, ssz], bf16, kind="ExternalInput")
    biasr = nc.dram_tensor("biasr", [P, NT], f32, kind="ExternalInput")
    out = nc.dram_tensor("out", [P, NT * BL], f32, kind="ExternalOutput")

    TG = 8  # tiles per S/W DMA chunk

    with tile.TileContext(nc) as tc:
        with (
            tc.tile_pool(name="const", bufs=1) as cpool,
            tc.tile_pool(name="fpool", bufs=3) as fpool,
            tc.tile_pool(name="upool", bufs=3) as upool,
            tc.tile_pool(name="psum", bufs=4, space="PSUM") as ppool,
        ):
            x_sb = cpool.tile([P, NPB, 2, BL, PBSZ], bf16)
            s_sb = cpool.tile([P, ssz], bf16)
            w_sb = cpool.tile([P, NT, 2, P], bf16)
            bias_sb = cpool.tile([P, NT], f32)
            z_sb = cpool.tile([P, NT, BL], f32)

            # DMA interleave: group g covers tiles [8g, 8g+8)
            pbdone = 0
            for g in range(NT // TG):
                t0, t1 = g * TG, (g + 1) * TG
                s0, s1 = soff[t0 // 2], soff[t1 // 2]
                nc.sync.dma_start(s_sb[:, s0:s1], ss[:, s0:s1])
                nc.sync.dma_start(
                    w_sb[:, t0:t1].rearrange("p t c n -> p (t c n)"),
                    ws[:, t0 * 2 * P : t1 * 2 * P],
                )
                pbneed = 1 + max(
                    r[0] for t in range(t0, t1) for bn in tiles[t] for r in bn
                )
                for pb in range(pbdone, pbneed):
                    nc.sync.dma_start(
                        x_sb[:, pb].rearrange("p c b q -> p (c b q)"),
                        xt[:, pb * 2 * BL * PBSZ : (pb + 1) * 2 * BL * PBSZ],
                    )
                pbdone = pbneed
            for pb in range(pbdone, NPB):
                nc.sync.dma_start(
                    x_sb[:, pb].rearrange("p c b q -> p (c b q)"),
                    xt[:, pb * 2 * BL * PBSZ : (pb + 1) * 2 * BL * PBSZ],
                )
            nc.sync.dma_start(bias_sb[:], biasr[:])

            # one-time memset of f pool buffers (pad columns must be finite;
            # stale SBUF could be NaN-patterned on first use)
            finit = []
            for _ in range(3):
                fb = fpool.tile([P, 2, BL, fdmax], bf16, tag="f")
                nc.gpsimd.memset(fb[:].rearrange("p a b c -> p (a b c)"), 0.0)
                finit.append(fb)

            # x view for moving operands: [P, pb, ch, b, row, x]
            x_r = x_sb[:].rearrange("p k c b (r x) -> p k c b r x", x=Wd)

            for q in range(NT // 2):  # tile pairs
                f_bf = fpool.tile([P, 2, BL, fdq[q]], bf16, tag="f")
                for tp in range(2):
                    t = 2 * q + tp
                    for bn in tiles[t]:
                        fd_bin = sum(r[2] * r[4] for r in bn)
                        bin0 = bn[0][5]
                        ps = ppool.tile([P, BL, 256], f32)
                        for ch in range(2):
                            for bp in range(2):
                                for ir, (br, rmin, nr, xmin, xl, coff) in enumerate(bn):
                                    o = coff - bin0
                                    nc.tensor.matmul(
                                        ps[:, 2 * bp : 2 * bp + 2, o : o + nr * xl],
                                        w_sb[:, t, ch, :],
                                        x_r[
                                            :, br, ch, 2 * bp : 2 * bp + 2,
                                            rmin - 8 * br : rmin - 8 * br + nr,
                                            xmin : xmin + xl,
                                        ],
                                        start=(ch == 0 and ir == 0),
                                        stop=(ch == 1 and ir == len(bn) - 1),
                                        skip_group_check=True,
                                    )
                        nc.scalar.copy(
                            f_bf[:, tp, :, bin0 : bin0 + fd_bin],
                            ps[:, :, 0:fd_bin],
                        )
                u = upool.tile([P, 2, BL, fdq[q]], bf16, tag="u")
                nc.vector.tensor_tensor(
                    out=u[:],
                    in0=f_bf[:],
                    in1=s_sb[:, soff[q] : soff[q + 1]]
                    .rearrange("p (t d) -> p t d", t=2)
                    .unsqueeze(2)
                    .broadcast_to([P, 2, BL, fdq[q]]),
                    op=mybir.AluOpType.mult,
                )
                nc.vector.tensor_reduce(
                    out=z_sb[:, 2 * q : 2 * q + 2],
                    in_=u[:],
                    axis=mybir.AxisListType.X,
                    op=mybir.AluOpType.add,
                )

            # epilogue: z += bias (broadcast over b); out = elu(z) + 1
            zf = cpool.tile([P, NT * BL], f32)
            ze = cpool.tile([P, NT * BL], f32)
            nc.vector.tensor_tensor(
                out=zf[:].rearrange("p (t b) -> p t b", b=BL),
                in0=z_sb[:],
                in1=bias_sb[:].unsqueeze(-1).broadcast_to([P, NT, BL]),
                op=mybir.AluOpType.add,
            )
            nc.vector.tensor_scalar_min(ze[:], zf[:], 0.0)
            nc.scalar.activation(ze[:], ze[:], mybir.ActivationFunctionType.Exp)
            nc.vector.tensor_scalar_max(zf[:], zf[:], 0.0)
            nc.vector.tensor_add(zf[:], zf[:], ze[:])
            nc.sync.dma_start(out[:], zf[:])

    from concourse.library_overlay import lower_extended_insts

    lower_extended_insts(nc)
    _split_multi_waits(nc)
    nc.finalize()
    return nc


def _split_multi_waits(nc):
    """The walrus build in this environment only supports ONE sync-wait slot
    per instruction.  Hoist extra waits onto NoOps inserted just before the
    offending instruction (same engine, so sequencer order enforces them)."""
    import concourse.mybir as mybir
    import bass_rust

    for fn in nc.m.functions:
        for blk in fn.blocks:
            new_insts = []
            for ins in blk.instructions:
                si = getattr(ins, "sync_info", None)
                waits = list(si.on_wait) if si is not None else []
                if len(waits) > 1:
                    for j, w in enumerate(waits[:-1]):
                        nop = mybir.InstNoOp(name=f"{ins.name}-w{j}")
                        nop.engine = ins.engine
                        nop.sync_info = bass_rust.SyncInfo(
                            on_wait=[w], on_update=[]
                        )
                        new_insts.append(nop)
                    ins.sync_info = bass_rust.SyncInfo(
                        on_wait=[waits[-1]], on_update=list(si.on_update)
                    )
                new_insts.append(ins)
            blk.instructions[:] = new_insts


def _host_prep(x, mu, W, b):
    bf16 = ml_dtypes.bfloat16

    # --- per-neuron bilinear indices / weights ---
    gx = np.clip(mu[:, 0].astype(np.float64), -1.0, 1.0)
    gy = np.clip(mu[:, 1].astype(np.float64), -1.0, 1.0)
    ix = (gx + 1.0) * (Wd * 0.5) - 0.5
    iy = (gy + 1.0) * (H * 0.5) - 0.5
    x0 = np.floor(ix)
    y0 = np.floor(iy)
    wx1 = (ix - x0).astype(np.float32)
    wy1 = (iy - y0).astype(np.float32)
    wx0 = 1.0 - wx1
    wy0 = 1.0 - wy1
    x0i = np.clip(x0.astype(np.int32), 0, Wd - 2)
    y0i = np.clip(y0.astype(np.int32), 0, H - 2)

    # sort by (block-row, block-col, y0, x0); 8x8 blocks
    order = np.lexsort((x0i, y0i, x0i // 8, y0i // 8))
    y0s, x0s = y0i[order], x0i[order]
    w4 = np.stack(
        [wx0 * wy0, wx1 * wy0, wx0 * wy1, wx1 * wy1], axis=-1
    ).astype(np.float32)[order]

    # --- per-tile rects (grouped by block-row), first-fit bins <= 256 cols ---
    tiles = []    # per tile: list of bins; bin = [(br, rmin, nr, xmin, xl, coloff)]
    fdt = []
    for t in range(NT):
        sl = slice(t * P, (t + 1) * P)
        yy, xx = y0s[sl], x0s[sl]
        rows = np.concatenate([yy, yy + 1])
        xs = np.concatenate([xx, xx])
        rd = {}
        for r, xc in zip(rows, xs):
            rd.setdefault(r // 8, []).append((r, xc))

        def make_rects(br, pts):
            rmin = min(p[0] for p in pts)
            rmax = max(p[0] for p in pts)
            xmin = min(p[1] for p in pts)
            xmax = max(p[1] for p in pts)
            nr = rmax - rmin + 1
            xl = xmax - xmin + 2
            xl += xl & 1  # pad to even for bf16 2x alignment
            if xl > Wd - xmin:
                xl = Wd - xmin  # halo fits (x0<=60) so only pad can overflow
            if nr * xl <= 256:
                return [(br, rmin, nr, xmin, xl)]
            # split at the largest x gap (fall back to median x)
            xsrt = sorted({p[1] for p in pts})
            gaps = [(xsrt[i + 1] - xsrt[i], xsrt[i]) for i in range(len(xsrt) - 1)]
            gmax = max(gaps)
            cut = gmax[1] if gmax[0] > 1 else xsrt[len(xsrt) // 2 - 1]
            lo = [p for p in pts if p[1] <= cut]
            hi = [p for p in pts if p[1] > cut]
            assert lo and hi, (t, br, cut)
            return make_rects(br, lo) + make_rects(br, hi)

        rects = []
        for br in sorted(rd):
            rects.extend(make_rects(br, rd[br]))
        # first-fit into bins of <= 256 cols
        bins = []
        for r in rects:
            sz = r[2] * r[4]
            for bn in bins:
                if bn[0] + sz <= 256:
                    bn[0] += sz
                    bn[1].append(r)
                    break
            else:
                bins.append([sz, [r]])
        # assign column offsets (contiguous across bins)
        col = 0
        obins = []
        for _, rs in bins:
            orl = []
            for (br, rmin, nr, xmin, xl) in rs:
                orl.append((br, rmin, nr, xmin, xl, col))
                col += nr * xl
            obins.append(orl)
        tiles.append(obins)
        fdt.append(col)

    # pair padding for rectangular DVE ops
    fdq = [max(fdt[2 * q], fdt[2 * q + 1]) for q in range(NT // 2)]
    fdq = [f + (f & 1) for f in fdq]
    soff = np.cumsum([0] + [2 * f for f in fdq]).tolist()
    ssz = soff[-1]

    # --- S' (bilinear weights over rect cols), pair-padded layout ---
    ss_np = np.zeros((P, ssz), dtype=np.float32)
    for t in range(NT):
        q, tp = t // 2, t % 2
        base = soff[q] + tp * fdq[q]
        sl = slice(t * P, (t + 1) * P)
        yy, xx, wg = y0s[sl], x0s[sl], w4[sl]
        allrects = [r for bn in tiles[t] for r in bn]
        for j in range(P):
            for (r, xc, wv) in (
                (yy[j], xx[j], wg[j, 0]),
                (yy[j], xx[j] + 1, wg[j, 1]),
                (yy[j] + 1, xx[j], wg[j, 2]),
                (yy[j] + 1, xx[j] + 1, wg[j, 3]),
            ):
                for (br, rmin, nr, xmin, xl, coff) in allrects:
                    if r // 8 == br and rmin <= r < rmin + nr \
                            and xmin <= xc < xmin + xl:
                        ss_np[j, base + coff + (r - rmin) * xl
                              + (xc - xmin)] += wv
                        break
                else:
                    raise AssertionError((t, j, r, xc))
    ss_np = ss_np.astype(bf16)

    # --- W stationary: [c_part, t, ch, n] ---
    Wp = W[order].astype(bf16)  # [N, C]
    ws_np = np.ascontiguousarray(
        Wp.reshape(NT, P, 2, P)        # [t, n, ch, c_part]
        .transpose(3, 0, 2, 1)         # [c_part, t, ch, n]
        .reshape(P, NT * 2 * P)
    )
    biasr_np = np.ascontiguousarray(b[order].astype(np.float32).reshape(NT, P).T)

    # --- per-core x channel-major: [c_part, pb, ch, b, pix-in-block] ---
    xb = x.astype(bf16).reshape(B, C, NPIX)
    xts = []
    for cix in range(NCORES):
        xc = xb[cix * BL : (cix + 1) * BL]              # [BL, C, NPIX]
        xt_np = np.ascontiguousarray(
            xc.reshape(BL, 2, P, NPB, PBSZ)             # [b, ch, cp, pb, q]
            .transpose(2, 3, 1, 0, 4)                   # [cp, pb, ch, b, q]
            .reshape(P, NPB * 2 * BL * PBSZ)
        )
        xts.append(xt_np)

    meta = {"tiles": tiles, "fdt": fdt, "fdq": fdq, "soff": soff, "ssz": ssz}
    shared = {"ss": ss_np, "ws": ws_np, "biasr": biasr_np}
    in_maps = [{"xt": xts[cix], **shared} for cix in range(NCORES)]
    return in_maps, meta, order


def _run(prep, trace=False, **kwargs):
    global _PROGRAM
    from concourse import bass_utils

    in_maps, meta, order = prep
    if _PROGRAM is None:
        _PROGRAM = _build_program(meta)
    rr = bass_utils.run_bass_kernel_spmd(
        _PROGRAM, in_maps, core_ids=list(range(NCORES)), trace=trace, **kwargs
    )
    inv = np.empty(N, dtype=np.int64)
    inv[order] = np.arange(N)
    outs = []
    for cix in range(NCORES):
        o = np.asarray(rr.results[cix]["out"], dtype=np.float32)  # [P, NT*BL]
        o = o.reshape(P, NT, BL).transpose(2, 1, 0).reshape(BL, N)  # sorted order
        outs.append(o[:, inv])
    return np.concatenate(outs, axis=0), rr


def kernel(x, mu, W, b):
    prep = _host_prep(x, mu, W, b)
    out, _ = _run(prep)
    return out


# revision 15
# speedup vs baseline: 5.0935x; 1.0886x over previous
"""Trainium2 Bass kernel for PoissonGaussianReadout.

Computation (per reference):
  out[b, n] = elu( sum_c bilinear_sample(x[b, c], mu[n]) * W[n, c] + bias[n] ) + 1

Sharding: data-parallel over batch B=32 across 8 cores (4 images per core).
Every core processes all N=8192 neurons for its 4 images.

Device strategy per core (v4 — "rect-B": channel dot on TensorE first,
then bilinear interpolation as a small DVE weighted sum):
  - Neurons are host-sorted by 8x8 spatial block of their receptive-field
    corner (y0//8, x0//8), packed into 64 tiles of 128.  A tile's corner
    pixels then span 1-2 small rectangles (~9x18 px, FD~152 avg).
  - x is host-transposed channel-major: x[c_part, pixblock, c_chunk, b, pix].
  - TensorE: Y[slot, b, rect-px] = sum_c W[slot, c] * x[c, b, rect-px] —
    stationary = per-tile W chunk [128c x 128n], moving = the rect pixels,
    PSUM-accumulated over the 2 c-chunks.  Per-b-pair matmuls keep each
    output inside one PSUM bank.
  - ScalarE drains PSUM f32 -> SBUF bf16 per bin (rects first-fit packed
    into <=256-col PSUM half-bank bins).
  - DVE: z[slot, b] = sum_px Y[slot, b, px] * S'[slot, px] where S' holds
    the 4 bilinear corner weights (zero elsewhere) — tensor_tensor mult
    (2x bf16) + tensor_reduce, per tile pair.
  - Epilogue: out = exp(min(z+bias,0)) + max(z+bias,0)  (== elu(z)+1).
"""

import numpy as np
import ml_dtypes

B, C, H, Wd, N = 32, 256, 64, 64, 8192
NCORES = 8
BL = B // NCORES          # 4 images per core
P = 128                   # partitions / neurons per tile
NT = N // P               # 64 neuron tiles
NPB = 8                   # pixel blocks (block-rows), 512 px each
NPIX = H * Wd             # 4096
PBSZ = 512                # pixels per block-row

_PROGRAM = None


def _build_program(meta):
    """meta: dict with per-tile rect/bin structure (see _host_prep)."""
    import concourse.bass as bass
    import concourse.mybir as mybir
    import concourse.tile as tile

    bf16 = mybir.dt.bfloat16
    f32 = mybir.dt.float32

    tiles = meta["tiles"]        # per tile: list of bins; bin = list of rects
                                 # rect = (br, rmin, nr, xmin, xl, coloff)
    fdt = meta["fdt"]            # per tile FD (cols)
    fdq = meta["fdq"]            # per pair padded FD
    soff = meta["soff"]          # per pair S' offset (elements per partition)
    ssz = meta["ssz"]
    fdmax = max(fdq)

    nc = bass.Bass("TRN2")

    xt = nc.dram_tensor("xt", [P, NPB * 2 * BL * PBSZ], bf16, kind="ExternalInput")
    ws = nc.dram_tensor("ws", [P, NT * 2 * P], bf16, kind="ExternalInput")
    ss = nc.dram_tensor("ss", [P, ssz], bf16, kind="ExternalInput")
    biasr = nc.dram_tensor("biasr", [P, NT], f32, kind="ExternalInput")
    out = nc.dram_tensor("out", [P, NT * BL], f32, kind="ExternalOutput")

    TG = 8  # tiles per S/W DMA chunk

    with tile.TileContext(nc) as tc:
        with (
            tc.tile_pool(name="const", bufs=1) as cpool,
            tc.tile_pool(name="fpool", bufs=3) as fpool,
            tc.tile_pool(name="upool", bufs=3) as upool,
            tc.tile_pool(name="psum", bufs=4, space="PSUM") as ppool,
        ):
            x_sb = cpool.tile([P, NPB, 2, BL, PBSZ], bf16)
            s_sb = cpool.tile([P, ssz], bf16)
            w_sb = cpool.tile([P, NT, 2, P], bf16)
            bias_sb = cpool.tile([P, NT], f32)
            z_sb = cpool.tile([P, NT, BL], f32)
            scr = cpool.tile([P, fdmax], bf16)

            # DMA interleave: fine-grained chunks early so tile-0 deps land
            # fast, coarser later.  Chunks are tile ranges.
            chunks = [(0, 2), (2, 4), (4, 8)] + [
                (t, t + TG) for t in range(8, NT, TG)
            ]
            pbdone = 0
            for t0, t1 in chunks:
                s0, s1 = soff[t0 // 2], soff[t1 // 2]
                nc.sync.dma_start(s_sb[:, s0:s1], ss[:, s0:s1])
                nc.sync.dma_start(
                    w_sb[:, t0:t1].rearrange("p t c n -> p (t c n)"),
                    ws[:, t0 * 2 * P : t1 * 2 * P],
                )
                pbneed = 1 + max(
                    r[0] for t in range(t0, t1) for bn in tiles[t] for r in bn
                )
                for pb in range(pbdone, pbneed):
                    nc.sync.dma_start(
                        x_sb[:, pb].rearrange("p c b q -> p (c b q)"),
                        xt[:, pb * 2 * BL * PBSZ : (pb + 1) * 2 * BL * PBSZ],
                    )
                pbdone = pbneed
            for pb in range(pbdone, NPB):
                nc.sync.dma_start(
                    x_sb[:, pb].rearrange("p c b q -> p (c b q)"),
                    xt[:, pb * 2 * BL * PBSZ : (pb + 1) * 2 * BL * PBSZ],
                )
            nc.sync.dma_start(bias_sb[:], biasr[:])

            # one-time memset of f pool buffers (pad columns must be finite;
            # stale SBUF could be NaN-patterned on first use)
            finit = []
            for _ in range(3):
                fb = fpool.tile([P, 2, BL, fdmax], bf16, tag="f")
                nc.gpsimd.memset(fb[:].rearrange("p a b c -> p (a b c)"), 0.0)
                finit.append(fb)

            # x view for moving operands: [P, pb, ch, b, row, x]
            x_r = x_sb[:].rearrange("p k c b (r x) -> p k c b r x", x=Wd)

            for q in range(NT // 2):  # tile pairs
                f_bf = fpool.tile([P, 2, BL, fdq[q]], bf16, tag="f")
                for tp in range(2):
                    t = 2 * q + tp
                    for bn in tiles[t]:
                        fd_bin = sum(r[2] * r[4] for r in bn)
                        bin0 = bn[0][5]
                        ps = ppool.tile([P, BL, 256], f32)
                        for ch in range(2):
                            for bp in range(2):
                                for ir, (br, rmin, nr, xmin, xl, coff) in enumerate(bn):
                                    o = coff - bin0
                                    nc.tensor.matmul(
                                        ps[:, 2 * bp : 2 * bp + 2, o : o + nr * xl],
                                        w_sb[:, t, ch, :],
                                        x_r[
                                            :, br, ch, 2 * bp : 2 * bp + 2,
                                            rmin - 8 * br : rmin - 8 * br + nr,
                                            xmin : xmin + xl,
                                        ],
                                        start=(ch == 0 and ir == 0),
                                        stop=(ch == 1 and ir == len(bn) - 1),
                                        skip_group_check=True,
                                    )
                        nc.scalar.copy(
                            f_bf[:, tp, :, bin0 : bin0 + fd_bin],
                            ps[:, :, 0:fd_bin],
                        )
                u = upool.tile([P, 2, BL, fdq[q]], bf16, tag="u")
                nc.vector.tensor_tensor(
                    out=u[:],
                    in0=f_bf[:],
                    in1=s_sb[:, soff[q] : soff[q + 1]]
                    .rearrange("p (t d) -> p t d", t=2)
                    .unsqueeze(2)
                    .broadcast_to([P, 2, BL, fdq[q]]),
                    op=mybir.AluOpType.mult,
                )
                if q % 2 == 1:
                    # balance engines: odd pairs reduce tile t1 on ScalarE
                    # (activation Copy with accum_out), t0 on DVE
                    nc.vector.tensor_reduce(
                        out=z_sb[:, 2 * q : 2 * q + 1],
                        in_=u[:, 0:1],
                        axis=mybir.AxisListType.X,
                        op=mybir.AluOpType.add,
                    )
                    for bb in range(BL):
                        nc.scalar.activation(
                            scr[:, 0 : fdq[q]],
                            u[:, 1, bb],
                            mybir.ActivationFunctionType.Copy,
                            accum_out=z_sb[:, 2 * q + 1, bb : bb + 1],
                        )
                else:
                    nc.vector.tensor_reduce(
                        out=z_sb[:, 2 * q : 2 * q + 2],
                        in_=u[:],
                        axis=mybir.AxisListType.X,
                        op=mybir.AluOpType.add,
                    )

            # epilogue: z += bias (broadcast over b); out = elu(z) + 1
            zf = cpool.tile([P, NT * BL], f32)
            ze = cpool.tile([P, NT * BL], f32)
            nc.vector.tensor_tensor(
                out=zf[:].rearrange("p (t b) -> p t b", b=BL),
                in0=z_sb[:],
                in1=bias_sb[:].unsqueeze(-1).broadcast_to([P, NT, BL]),
                op=mybir.AluOpType.add,
            )
            nc.vector.tensor_scalar_min(ze[:], zf[:], 0.0)
            nc.scalar.activation(ze[:], ze[:], mybir.ActivationFunctionType.Exp)
            nc.vector.tensor_scalar_max(zf[:], zf[:], 0.0)
            nc.vector.tensor_add(zf[:], zf[:], ze[:])
            nc.sync.dma_start(out[:], zf[:])

    from concourse.library_overlay import lower_extended_insts

    lower_extended_insts(nc)
    _split_multi_waits(nc)
    nc.finalize()
    return nc


def _split_multi_waits(nc):
    """The walrus build in this environment only supports ONE sync-wait slot
    per instruction.  Hoist extra waits onto NoOps inserted just before the
    offending instruction (same engine, so sequencer order enforces them)."""
    import concourse.mybir as mybir
    import bass_rust

    for fn in nc.m.functions:
        for blk in fn.blocks:
            new_insts = []
            for ins in blk.instructions:
                si = getattr(ins, "sync_info", None)
                waits = list(si.on_wait) if si is not None else []
                if len(waits) > 1:
                    for j, w in enumerate(waits[:-1]):
                        nop = mybir.InstNoOp(name=f"{ins.name}-w{j}")
                        nop.engine = ins.engine
                        nop.sync_info = bass_rust.SyncInfo(
                            on_wait=[w], on_update=[]
                        )
                        new_insts.append(nop)
                    ins.sync_info = bass_rust.SyncInfo(
                        on_wait=[waits[-1]], on_update=list(si.on_update)
                    )
                new_insts.append(ins)
            blk.instructions[:] = new_insts


def _host_prep(x, mu, W, b):
    bf16 = ml_dtypes.bfloat16

    # --- per-neuron bilinear indices / weights ---
    gx = np.clip(mu[:, 0].astype(np.float64), -1.0, 1.0)
    gy = np.clip(mu[:, 1].astype(np.float64), -1.0, 1.0)
    ix = (gx + 1.0) * (Wd * 0.5) - 0.5
    iy = (gy + 1.0) * (H * 0.5) - 0.5
    x0 = np.floor(ix)
    y0 = np.floor(iy)
    wx1 = (ix - x0).astype(np.float32)
    wy1 = (iy - y0).astype(np.float32)
    wx0 = 1.0 - wx1
    wy0 = 1.0 - wy1
    x0i = np.clip(x0.astype(np.int32), 0, Wd - 2)
    y0i = np.clip(y0.astype(np.int32), 0, H - 2)

    # sort by (block-row, x0, y0): tiles become narrow x-windows within an
    # 8-row band -> mostly one small rect each
    order = np.lexsort((y0i, x0i, y0i // 8))
    y0s, x0s = y0i[order], x0i[order]
    w4 = np.stack(
        [wx0 * wy0, wx1 * wy0, wx0 * wy1, wx1 * wy1], axis=-1
    ).astype(np.float32)[order]

    # --- per-tile rects (grouped by block-row), first-fit bins <= 256 cols ---
    tiles = []    # per tile: list of bins; bin = [(br, rmin, nr, xmin, xl, coloff)]
    fdt = []
    for t in range(NT):
        sl = slice(t * P, (t + 1) * P)
        yy, xx = y0s[sl], x0s[sl]
        rows = np.concatenate([yy, yy + 1])
        xs = np.concatenate([xx, xx])
        rd = {}
        for r, xc in zip(rows, xs):
            rd.setdefault(r // 8, []).append((r, xc))

        def make_rects(br, pts):
            rmin = min(p[0] for p in pts)
            rmax = max(p[0] for p in pts)
            xmin = min(p[1] for p in pts)
            xmax = max(p[1] for p in pts)
            nr = rmax - rmin + 1
            xl = xmax - xmin + 2
            xl += xl & 1  # pad to even for bf16 2x alignment
            if xl > Wd - xmin:
                xl = Wd - xmin  # halo fits (x0<=60) so only pad can overflow
            if nr * xl <= 256:
                return [(br, rmin, nr, xmin, xl)]
            # split at the largest x gap (fall back to median x)
            xsrt = sorted({p[1] for p in pts})
            gaps = [(xsrt[i + 1] - xsrt[i], xsrt[i]) for i in range(len(xsrt) - 1)]
            gmax = max(gaps)
            cut = gmax[1] if gmax[0] > 1 else xsrt[len(xsrt) // 2 - 1]
            lo = [p for p in pts if p[1] <= cut]
            hi = [p for p in pts if p[1] > cut]
            assert lo and hi, (t, br, cut)
            return make_rects(br, lo) + make_rects(br, hi)

        rects = []
        for br in sorted(rd):
            rects.extend(make_rects(br, rd[br]))
        # first-fit into bins of <= 256 cols
        bins = []
        for r in rects:
            sz = r[2] * r[4]
            for bn in bins:
                if bn[0] + sz <= 256:
                    bn[0] += sz
                    bn[1].append(r)
                    break
            else:
                bins.append([sz, [r]])
        # assign column offsets (contiguous across bins)
        col = 0
        obins = []
        for _, rs in bins:
            orl = []
            for (br, rmin, nr, xmin, xl) in rs:
                orl.append((br, rmin, nr, xmin, xl, col))
                col += nr * xl
            obins.append(orl)
        tiles.append(obins)
        fdt.append(col)

    # pair padding for rectangular DVE ops
    fdq = [max(fdt[2 * q], fdt[2 * q + 1]) for q in range(NT // 2)]
    fdq = [f + (f & 1) for f in fdq]
    soff = np.cumsum([0] + [2 * f for f in fdq]).tolist()
    ssz = soff[-1]

    # --- S' (bilinear weights over rect cols), pair-padded layout ---
    ss_np = np.zeros((P, ssz), dtype=np.float32)
    for t in range(NT):
        q, tp = t // 2, t % 2
        base = soff[q] + tp * fdq[q]
        sl = slice(t * P, (t + 1) * P)
        yy, xx, wg = y0s[sl], x0s[sl], w4[sl]
        allrects = [r for bn in tiles[t] for r in bn]
        for j in range(P):
            for (r, xc, wv) in (
                (yy[j], xx[j], wg[j, 0]),
                (yy[j], xx[j] + 1, wg[j, 1]),
                (yy[j] + 1, xx[j], wg[j, 2]),
                (yy[j] + 1, xx[j] + 1, wg[j, 3]),
            ):
                for (br, rmin, nr, xmin, xl, coff) in allrects:
                    if r // 8 == br and rmin <= r < rmin + nr \
                            and xmin <= xc < xmin + xl:
                        ss_np[j, base + coff + (r - rmin) * xl
                              + (xc - xmin)] += wv
                        break
                else:
                    raise AssertionError((t, j, r, xc))
    ss_np = ss_np.astype(bf16)

    # --- W stationary: [c_part, t, ch, n] ---
    Wp = W[order].astype(bf16)  # [N, C]
    ws_np = np.ascontiguousarray(
        Wp.reshape(NT, P, 2, P)        # [t, n, ch, c_part]
        .transpose(3, 0, 2, 1)         # [c_part, t, ch, n]
        .reshape(P, NT * 2 * P)
    )
    biasr_np = np.ascontiguousarray(b[order].astype(np.float32).reshape(NT, P).T)

    # --- per-core x channel-major: [c_part, pb, ch, b, pix-in-block] ---
    xb = x.astype(bf16).reshape(B, C, NPIX)
    xts = []
    for cix in range(NCORES):
        xc = xb[cix * BL : (cix + 1) * BL]              # [BL, C, NPIX]
        xt_np = np.ascontiguousarray(
            xc.reshape(BL, 2, P, NPB, PBSZ)             # [b, ch, cp, pb, q]
            .transpose(2, 3, 1, 0, 4)                   # [cp, pb, ch, b, q]
            .reshape(P, NPB * 2 * BL * PBSZ)
        )
        xts.append(xt_np)

    meta = {"tiles": tiles, "fdt": fdt, "fdq": fdq, "soff": soff, "ssz": ssz}
    shared = {"ss": ss_np, "ws": ws_np, "biasr": biasr_np}
    in_maps = [{"xt": xts[cix], **shared} for cix in range(NCORES)]
    return in_maps, meta, order


def _run(prep, trace=False, **kwargs):
    global _PROGRAM
    from concourse import bass_utils

    in_maps, meta, order = prep
    if _PROGRAM is None:
        _PROGRAM = _build_program(meta)
    rr = bass_utils.run_bass_kernel_spmd(
        _PROGRAM, in_maps, core_ids=list(range(NCORES)), trace=trace, **kwargs
    )
    inv = np.empty(N, dtype=np.int64)
    inv[order] = np.arange(N)
    outs = []
    for cix in range(NCORES):
        o = np.asarray(rr.results[cix]["out"], dtype=np.float32)  # [P, NT*BL]
        o = o.reshape(P, NT, BL).transpose(2, 1, 0).reshape(BL, N)  # sorted order
        outs.append(o[:, inv])
    return np.concatenate(outs, axis=0), rr


def kernel(x, mu, W, b):
    prep = _host_prep(x, mu, W, b)
    out, _ = _run(prep)
    return out
